# revision 8
# baseline (speedup 1.0000x reference)
"""Trainium2 Bass kernel for nn_DecoderRNN (attention LSTM decoder).

Sharding (8 cores):
  - attention (tanh/softmax DVE/ACT work + context einsum): batch-sharded (8/core)
  - LSTM gates matmul: gate-column-sharded (512 cols/core, 128 per gate);
    h/c column-sharded 128/core
  - fc_out: vocab-sharded (4000 cols/core)
Per step two AllGather exchanges: context [64,2048] and hT [1024,64].
Softmax normalization is deferred: the context einsum uses unnormalized
exp(scores); the 1/sum scale is applied at the context psum evacuation and
attention weights are normalized on the host.

Host does: slicing/padding/transposes/casts of inputs, embedding gather,
final output assembly. All FLOPs run on device.
"""
import sys
sys.path.insert(0, "/opt/trn_rl_repo")
from contextlib import ExitStack
import numpy as np
import ml_dtypes

bf16 = ml_dtypes.bfloat16

import concourse.bass as bass
import concourse.bacc as bacc
import concourse.mybir as mybir
from concourse import tile
from concourse.bass_utils import run_bass_kernel_spmd

F32 = mybir.dt.float32
BF16 = mybir.dt.bfloat16
ADD = mybir.AluOpType.add
MUL = mybir.AluOpType.mult
TANH = mybir.ActivationFunctionType.Tanh
EXP = mybir.ActivationFunctionType.Exp

NC = 8                     # cores
B, NF, T = 64, 196, 32     # batch, n_feat, caption len
TS = T - 1                 # 31 decode steps
EMB, VOCAB, ATTN, ENC_H, DEC_H = 512, 32000, 512, 2048, 1024
BL = B // NC               # 8 local batches
CS = 4 * DEC_H // NC       # 512 gate cols per core (128 per gate)
HS = DEC_H // NC           # 128 h cols per core
VS = VOCAB // NC           # 4000 vocab cols per core
NR, NFo = 16, 16           # n = nr*16 + nf (pad 196 -> 256); partition p = nf*8+bl
AFREE = NR * ATTN          # 8192 attention free size

_nc_cache = None


def build():
    global _nc_cache
    if _nc_cache is not None:
        return _nc_cache
    nc = bacc.Bacc("TRN2", target_bir_lowering=False, debug=False, num_devices=NC)
    tc = tile.TileContext(nc)

    def inp(name, shape, dtype=BF16):
        return nc.declare_dram_parameter(name, shape, dtype, isOutput=False)

    f_ein = inp("f_ein", [128, 16 * ENC_H])          # einsum rhs tiles
    fT_aug = inp("fT_aug", [17 * 128, 2048])         # F^T (perm cols) + bias block
    W_u_aug = inp("W_u_aug", [17 * 128, ATTN])
    embT_aug = inp("embT_aug", [5 * 128, 2048])      # emb^T ((t,b) cols) + bias block
    W_top_aug = inp("W_top_aug", [5 * 128, CS])
    W_mid = inp("W_mid", [ENC_H, CS])                # W_ih[512:, cols]
    W_hh = inp("W_hh", [DEC_H, CS])
    W_w = inp("W_w", [DEC_H, ATTN])
    W_a_rep = inp("W_a_rep", [128, ATTN])
    W_h0_aug = inp("W_h0_aug", [17 * 128, HS])
    W_c0_aug = inp("W_c0_aug", [17 * 128, HS])
    w_fc = inp("w_fc", [DEC_H, VS])
    mean_blk = inp("mean_blk", [128, 128])           # 1/NF blockdiag for feat-mean
    bsel = inp("bsel", [128, 128])                   # local-batch selector (pad rows 0)
    ones_blk = inp("ones_blk", [128, 8])             # group-sum lhsT
    onesb = inp("onesb", [128, B])                   # row0=1 block (bias K-tile lhsT)
    mask = inp("mask", [128, NR])                    # 1.0 on valid n slots
    zer = inp("zer", [128, 512])                     # zeros
    ident = inp("ident", [128, 128])

    preds_part = nc.declare_dram_parameter("preds_part", [2048, VS], F32, isOutput=True)
    attn_scr = nc.declare_dram_parameter("attn_scr", [TS, 128, NR], F32, isOutput=True)
    rec_scr = nc.declare_dram_parameter("rec_scr", [TS, BL, 1], F32, isOutput=True)

    gr = [list(range(NC))]

    with ExitStack() as ctx:
        ctx.enter_context(tc)
        # ---------- persistent pools ----------
        P = ctx.enter_context(tc.tile_pool(name="persist", bufs=1))
        D = ctx.enter_context(tc.tile_pool(name="drams", bufs=1, space="DRAM"))

        fein_sb = P.tile([128, 16 * ENC_H], BF16)
        Uf = P.tile([128, AFREE], BF16)
        g0 = P.tile([128, 16 * CS], BF16)            # gates0, rows (t,b) 2048pad
        Wmid_sb = P.tile([128, 16 * CS], BF16)
        Whh_sb = P.tile([128, 8 * CS], BF16)
        Ww_sb = P.tile([128, 8 * ATTN], BF16)
        Wa_sb = P.tile([128, ATTN], BF16)
        bsel_sb = P.tile([128, 128], BF16)
        ob_sb = P.tile([128, 8], BF16)
        onesb_sb = P.tile([128, B], BF16)
        mask_sb = P.tile([128, NR], BF16)
        id_sb = P.tile([128, 128], BF16)
        Ablk = P.tile([128, 16 * 8], BF16)           # einsum lhsT blockdiag
        hT_full = P.tile([128, 8 * B], BF16)         # [128,(8kt,64b)]
        pre_all = P.tile([128, ATTN], BF16)          # rows 0:64 written per step
        c_cs = P.tile([B, HS], F32)

        ctx_shard = D.tile([BL, ENC_H], BF16, tag="ctx_shard")
        ctx_full = D.tile([B, ENC_H], BF16, tag="ctx_full")
        hT_shard = D.tile([HS, B], BF16, tag="hT_shard")
        hT_dram = D.tile([(TS + 2) * DEC_H, B], BF16, tag="hT_dram")
        fm_shard = D.tile([BL, ENC_H], BF16, tag="fm_shard")
        fm_full = D.tile([B, ENC_H], BF16, tag="fm_full")

        nc.sync.dma_start(fein_sb[:], f_ein[:])
        nc.sync.dma_start(Wmid_sb[:].rearrange("p (k c) -> p k c", k=16),
                          W_mid.rearrange("(k p) c -> p k c", p=128))
        nc.sync.dma_start(Whh_sb[:].rearrange("p (k c) -> p k c", k=8),
                          W_hh.rearrange("(k p) c -> p k c", p=128))
        nc.sync.dma_start(Ww_sb[:].rearrange("p (k c) -> p k c", k=8),
                          W_w.rearrange("(k p) c -> p k c", p=128))
        nc.sync.dma_start(Wa_sb[:], W_a_rep[:])
        nc.sync.dma_start(bsel_sb[:], bsel[:])
        nc.sync.dma_start(ob_sb[:], ones_blk[:])
        nc.sync.dma_start(onesb_sb[:], onesb[:])
        nc.sync.dma_start(mask_sb[:], mask[:])
        nc.sync.dma_start(id_sb[:], ident[:])
        nc.sync.dma_start(Ablk[:], zer[:, 0:128])
        nc.sync.dma_start(pre_all[64:128, :], zer[0:64, :])

        # ---------- phase 0 ----------
        with tc.tile_pool(name="ph0", bufs=2) as p0, \
             tc.tile_pool(name="ph0w", bufs=1) as p0w, \
             tc.tile_pool(name="ph0ps", bufs=1, space="PSUM") as pp0:
            def acc(i, shape=(128, 512), dt=F32):
                return pp0.tile(list(shape), dt, tag=f"acc{i}", name=f"acc{i}")

            # gates0 = embT_aug^T @ W_top_aug  (rows (t,b), cols CS)
            wtop = p0w.tile([128, 5 * CS], BF16, tag="wtop")
            nc.sync.dma_start(wtop[:].rearrange("p (k c) -> p k c", k=5),
                              W_top_aug.rearrange("(k p) c -> p k c", p=128))
            for half in range(2):
                psg = [acc(m) for m in range(8)]
                for k in range(5):
                    et = p0.tile([128, 1024], BF16, tag="ldA", name="ldA")
                    nc.sync.dma_start(et[:], embT_aug[k * 128:(k + 1) * 128,
                                                      half * 1024:(half + 1) * 1024])
                    for m in range(8):
                        nc.tensor.matmul(psg[m][:], et[:, m * 128:(m + 1) * 128],
                                         wtop[:, k * CS:(k + 1) * CS],
                                         start=(k == 0), stop=(k == 4))
                for m in range(8):
                    mt = half * 8 + m
                    nc.vector.tensor_copy(g0[:, mt * CS:(mt + 1) * CS], psg[m][:])

            # Uf = fT_aug^T @ W_u_aug  (m-tile == nr chunk)
            for half in range(2):
                psu = [acc(m) for m in range(8)]
                for k in range(17):
                    ft = p0.tile([128, 1024], BF16, tag="ldA", name="ldA")
                    nc.sync.dma_start(ft[:], fT_aug[k * 128:(k + 1) * 128,
                                                    half * 1024:(half + 1) * 1024])
                    wu = p0.tile([128, ATTN], BF16, tag="ldB", name="ldB")
                    nc.sync.dma_start(wu[:], W_u_aug[k * 128:(k + 1) * 128, :])
                    for m in range(8):
                        nc.tensor.matmul(psu[m][:], ft[:, m * 128:(m + 1) * 128],
                                         wu[:], start=(k == 0), stop=(k == 16))
                for m in range(8):
                    nr = half * 8 + m
                    nc.vector.tensor_copy(Uf[:, nr * ATTN:(nr + 1) * ATTN], psu[m][:])

            # feat mean via einsum with mean_blk
            mb = p0.tile([128, 128], BF16, tag="ldB", name="ldB")
            nc.sync.dma_start(mb[:], mean_blk[:])
            psf = [acc(c, (BL, 512)) for c in range(4)]
            for kt in range(16):
                for c in range(4):
                    nc.tensor.matmul(psf[c][:], mb[:, kt * 8:(kt + 1) * 8],
                                     fein_sb[:, kt * ENC_H + c * 512:kt * ENC_H + (c + 1) * 512],
                                     start=(kt == 0), stop=(kt == 15))
            fmb = p0.tile([BL, ENC_H], BF16, tag="fmb")
            for c in range(4):
                nc.vector.tensor_copy(fmb[:, c * 512:(c + 1) * 512], psf[c][:])
            nc.sync.dma_start(fm_shard[:], fmb[:])
            nc.gpsimd.collective_compute("AllGather", mybir.AluOpType.bypass,
                                         replica_groups=gr, ins=[fm_shard[:].opt()],
                                         outs=[fm_full[:].opt()])
            fmf = p0.tile([B, ENC_H], BF16, tag="fmf")
            nc.sync.dma_start(fmf[:], fm_full[:])
            fmT = p0.tile([128, 16 * B], BF16, tag="fmT")
            for k in range(16):
                pst = acc(4, (128, B), BF16)
                nc.tensor.transpose(pst[:], fmf[:, k * 128:(k + 1) * 128], id_sb[:B, :B])
                nc.vector.tensor_copy(fmT[:, k * B:(k + 1) * B], pst[:])
            # h0/c0 col slices
            wh0 = p0w.tile([128, 17 * HS], BF16, tag="wh0")
            wc0 = p0w.tile([128, 17 * HS], BF16, tag="wc0")
            nc.sync.dma_start(wh0[:].rearrange("p (k c) -> p k c", k=17),
                              W_h0_aug.rearrange("(k p) c -> p k c", p=128))
            nc.sync.dma_start(wc0[:].rearrange("p (k c) -> p k c", k=17),
                              W_c0_aug.rearrange("(k p) c -> p k c", p=128))
            ph0 = acc(0, (B, HS))
            pc0 = acc(1, (B, HS))
            for k in range(17):
                lh = fmT[:, k * B:(k + 1) * B] if k < 16 else onesb_sb[:]
                nc.tensor.matmul(ph0[:], lh, wh0[:, k * HS:(k + 1) * HS],
                                 start=(k == 0), stop=(k == 16))
                nc.tensor.matmul(pc0[:], lh, wc0[:, k * HS:(k + 1) * HS],
                                 start=(k == 0), stop=(k == 16))
            nc.vector.tensor_copy(c_cs[:], pc0[:])
            h_bf0 = p0.tile([B, HS], BF16, tag="hbf0")
            nc.vector.tensor_copy(h_bf0[:], ph0[:])
            psT0 = acc(2, (HS, B), BF16)
            nc.tensor.transpose(psT0[:], h_bf0[:], id_sb[:B, :B])
            hsh0 = p0.tile([HS, B], BF16, tag="hsh0")
            nc.vector.tensor_copy(hsh0[:], psT0[:])
            nc.sync.dma_start(hT_shard[:], hsh0[:])
            nc.gpsimd.collective_compute("AllGather", mybir.AluOpType.bypass,
                                         replica_groups=gr, ins=[hT_shard[:].opt()],
                                         outs=[hT_dram[0:DEC_H, :].opt()])
            nc.sync.dma_start(
                hT_full[:].rearrange("p (k b) -> p k b", k=8),
                hT_dram[0:DEC_H, :].rearrange("(k p) b -> p k b", p=128))

        # ---------- phase 1: 31 steps ----------
        with tc.tile_pool(name="step", bufs=2) as sp, \
             tc.tile_pool(name="stepps", bufs=3, space="PSUM") as ps, \
             tc.tile_pool(name="ctxps", bufs=1, space="PSUM") as psc:
            for t in range(TS):
                # --- pre = h @ W_w for ALL b, then select+replicate local 8 ---
                pre_ps = ps.tile([B, ATTN], F32, tag="mm", name="pre_ps")
                for k in range(8):
                    nc.tensor.matmul(pre_ps[:], hT_full[:, k * B:(k + 1) * B],
                                     Ww_sb[:, k * ATTN:(k + 1) * ATTN],
                                     start=(k == 0), stop=(k == 7))
                nc.vector.tensor_copy(pre_all[0:B, :], pre_ps[:])
                prr_ps = ps.tile([128, ATTN], F32, tag="mm", name="prr_ps")
                nc.tensor.matmul(prr_ps[:], bsel_sb[:], pre_all[:], start=True, stop=True)
                pre_rep = sp.tile([128, ATTN], BF16, tag="pre_rep")
                nc.vector.tensor_copy(pre_rep[:], prr_ps[:])
                # --- att = tanh(Uf + pre) ; scores = sum_a att*W_a ---
                att = sp.tile([128, AFREE], BF16, tag="att")
                nc.vector.tensor_tensor(
                    att[:].rearrange("p (r a) -> p r a", r=NR),
                    Uf[:].rearrange("p (r a) -> p r a", r=NR),
                    bass.AP(pre_rep[:].tensor, 0, [[ATTN, 128], [0, NR], [1, ATTN]]),
                    ADD)
                nc.scalar.activation(att[:], att[:], TANH)
                nc.vector.tensor_tensor(
                    att[:].rearrange("p (r a) -> p r a", r=NR),
                    att[:].rearrange("p (r a) -> p r a", r=NR),
                    bass.AP(Wa_sb[:].tensor, 0, [[ATTN, 128], [0, NR], [1, ATTN]]),
                    MUL)
                scores = sp.tile([128, NR], F32, tag="scores")
                nc.vector.tensor_reduce(scores[:],
                                        att[:].rearrange("p (r a) -> p r a", r=NR),
                                        mybir.AxisListType.X, ADD)
                # --- masked exp; deferred softmax normalization ---
                expv = sp.tile([128, NR], F32, tag="expv")
                nc.scalar.activation(expv[:], scores[:], EXP)
                exb = sp.tile([128, NR], BF16, tag="exb")
                nc.vector.tensor_tensor(exb[:], expv[:], mask_sb[:], MUL)
                ssum = ps.tile([BL, NR], F32, tag="mm", name="ssum")
                nc.tensor.matmul(ssum[:], ob_sb[:], exb[:], start=True, stop=True)
                ssum_sb = sp.tile([BL, NR], F32, tag="ssum_sb")
                nc.vector.tensor_copy(ssum_sb[:], ssum[:])
                stot = sp.tile([BL, 1], F32, tag="stot")
                nc.vector.tensor_reduce(stot[:], ssum_sb[:], mybir.AxisListType.X, ADD)
                rec = sp.tile([BL, 1], F32, tag="rec")
                nc.vector.reciprocal(rec[:], stot[:])
                nc.sync.dma_start(attn_scr[t], expv[:])
                nc.sync.dma_start(rec_scr[t], rec[:])
                # --- exp -> blockdiag lhsT (transpose + 2 scatter DMAs) ---
                aT_ps = ps.tile([NR, 128], BF16, tag="mm", name="aT_ps")
                nc.tensor.transpose(aT_ps[:], exb[:], id_sb[:])
                aT_sb = sp.tile([NR, 128], BF16, tag="aT_sb")
                nc.vector.tensor_copy(aT_sb[:], aT_ps[:])
                for h in range(2):
                    asrc = bass.AP(aT_sb[:].tensor, 8 * h * 128,
                                   [[128, 8], [8, NFo], [1, 8]])
                    adst = bass.AP(Ablk[:].tensor, 8 * h, [[128, 128], [17, 8]])
                    nc.sync.dma_start(adst, asrc)
                # --- context einsum (unnormalized), scaled on evacuation ---
                ctx_ps = psc.tile([BL, ENC_H], F32, tag="ctx_ps", name="ctx_ps")
                for kt in range(16):
                    for c in range(4):
                        nc.tensor.matmul(
                            ctx_ps[:, c * 512:(c + 1) * 512],
                            Ablk[:, kt * 8:(kt + 1) * 8],
                            fein_sb[:, kt * ENC_H + c * 512:kt * ENC_H + (c + 1) * 512],
                            start=(kt == 0), stop=(kt == 15))
                ctx_bf = sp.tile([BL, ENC_H], BF16, tag="ctx_bf")
                for c in range(2):
                    nc.vector.tensor_scalar(ctx_bf[:, c * 1024:(c + 1) * 1024],
                                            ctx_ps[:, c * 1024:(c + 1) * 1024],
                                            rec[:], None, MUL)
                nc.sync.dma_start(ctx_shard[:], ctx_bf[:])
                nc.gpsimd.collective_compute("AllGather", mybir.AluOpType.bypass,
                                             replica_groups=gr, ins=[ctx_shard[:].opt()],
                                             outs=[ctx_full[:].opt()])
                ctxf = sp.tile([B, ENC_H], BF16, tag="ctxf")
                nc.sync.dma_start(ctxf[:], ctx_full[:])
                ctxT = sp.tile([128, 16 * B], BF16, tag="ctxT")
                for k4 in range(4):
                    cT_ps = ps.tile([128, 4 * B], BF16, tag="mm", name="cT_ps")
                    for j in range(4):
                        kk = k4 * 4 + j
                        nc.tensor.transpose(cT_ps[:, j * B:(j + 1) * B],
                                            ctxf[:, kk * 128:(kk + 1) * 128],
                                            id_sb[:B, :B])
                    nc.vector.tensor_copy(ctxT[:, k4 * 4 * B:(k4 + 1) * 4 * B], cT_ps[:])
                # --- gates ---
                g_ps = ps.tile([B, CS], F32, tag="mm", name="g_ps")
                for k in range(16):
                    nc.tensor.matmul(g_ps[:], ctxT[:, k * B:(k + 1) * B],
                                     Wmid_sb[:, k * CS:(k + 1) * CS],
                                     start=(k == 0), stop=False)
                for k in range(8):
                    nc.tensor.matmul(g_ps[:], hT_full[:, k * B:(k + 1) * B],
                                     Whh_sb[:, k * CS:(k + 1) * CS],
                                     start=False, stop=(k == 7))
                gat = sp.tile([B, CS], F32, tag="gat")
                nc.vector.tensor_tensor(gat[:], g_ps[:],
                                        g0[(t % 2) * 64:(t % 2) * 64 + 64,
                                           (t // 2) * CS:(t // 2 + 1) * CS], ADD)
                # --- elementwise: col chunks (i, f, o, g) of 128 ---
                sig = sp.tile([B, 384], F32, tag="sig")
                nc.scalar.activation(sig[:], gat[:, 0:384], TANH, scale=0.5)
                nc.vector.tensor_scalar(sig[:], sig[:], 0.5, 0.5, MUL, ADD)
                tg = sp.tile([B, HS], F32, tag="tg")
                nc.scalar.activation(tg[:], gat[:, 384:512], TANH)
                m1 = sp.tile([B, HS], F32, tag="m1")
                nc.vector.tensor_tensor(m1[:], sig[:, 128:256], c_cs[:], MUL)
                m2 = sp.tile([B, HS], F32, tag="m2")
                nc.vector.tensor_tensor(m2[:], sig[:, 0:128], tg[:], MUL)
                nc.vector.tensor_tensor(c_cs[:], m1[:], m2[:], ADD)
                tcn = sp.tile([B, HS], F32, tag="tcn")
                nc.scalar.activation(tcn[:], c_cs[:], TANH)
                h_bf = sp.tile([B, HS], BF16, tag="h_bf")
                nc.vector.tensor_tensor(h_bf[:], sig[:, 256:384], tcn[:], MUL)
                # --- h exchange ---
                hT_ps = ps.tile([HS, B], BF16, tag="mm", name="hT_ps")
                nc.tensor.transpose(hT_ps[:], h_bf[:], id_sb[:B, :B])
                hsh = sp.tile([HS, B], BF16, tag="hsh")
                nc.vector.tensor_copy(hsh[:], hT_ps[:])
                nc.sync.dma_start(hT_shard[:], hsh[:])
                nc.gpsimd.collective_compute(
                    "AllGather", mybir.AluOpType.bypass, replica_groups=gr,
                    ins=[hT_shard[:].opt()],
                    outs=[hT_dram[(t + 1) * DEC_H:(t + 2) * DEC_H, :].opt()])
                nc.sync.dma_start(
                    hT_full[:].rearrange("p (k b) -> p k b", k=8),
                    hT_dram[(t + 1) * DEC_H:(t + 2) * DEC_H, :]
                    .rearrange("(k p) b -> p k b", p=128))

        # ---------- fc phase ----------
        with tc.tile_pool(name="fc", bufs=3) as fp, \
             tc.tile_pool(name="fcps", bufs=1, space="PSUM") as fps:
            NVC = 8
            VC = VS // NVC  # 500
            hT_view = hT_dram[:].rearrange("(t p) b -> t p b", p=DEC_H)
            for half in range(2):
                for vc in range(NVC):
                    psv = [fps.tile([128, VC], F32, tag=f"psv{m}", name=f"psv{m}")
                           for m in range(8)]
                    for k in range(8):
                        wfc = fp.tile([128, VC], BF16, tag="wfc", name="wfc")
                        nc.sync.dma_start(wfc[:], w_fc[k * 128:(k + 1) * 128,
                                                       vc * VC:(vc + 1) * VC])
                        for m in range(8):
                            mt = half * 8 + m
                            ht = fp.tile([128, 128], BF16, tag="ht", name="ht")
                            nc.sync.dma_start(
                                ht[:].rearrange("p (t b) -> p t b", t=2),
                                hT_view[2 * mt + 1:2 * mt + 3,
                                        k * 128:(k + 1) * 128, :]
                                .rearrange("t p b -> p t b"))
                            nc.tensor.matmul(psv[m][:], ht[:], wfc[:],
                                             start=(k == 0), stop=(k == 7))
                    for m in range(8):
                        mt = half * 8 + m
                        ev = fp.tile([128, VC], F32, tag="ev", name="ev")
                        if m % 2 == 0:
                            nc.vector.tensor_copy(ev[:], psv[m][:])
                        else:
                            nc.scalar.copy(ev[:], psv[m][:])
                        nc.sync.dma_start(preds_part[mt * 128:(mt + 1) * 128,
                                                     vc * VC:(vc + 1) * VC], ev[:])
    nc.finalize()
    _nc_cache = nc
    return nc


def host_prep(inputs):
    f32 = np.float32
    features = np.asarray(inputs["features"], f32)       # [64,196,2048]
    captions = np.asarray(inputs["captions"])            # [64,32] int
    emb_table = np.asarray(inputs["emb_table"], f32)
    W_u = np.asarray(inputs["W_u"], f32); b_u = np.asarray(inputs["b_u"], f32)
    W_w = np.asarray(inputs["W_w"], f32); b_w = np.asarray(inputs["b_w"], f32)
    W_a = np.asarray(inputs["W_a"], f32)
    W_ih = np.asarray(inputs["W_ih"], f32); b_ih = np.asarray(inputs["b_ih"], f32)
    W_hh = np.asarray(inputs["W_hh"], f32); b_hh = np.asarray(inputs["b_hh"], f32)
    W_fc = np.asarray(inputs["W_fc"], f32)
    W_h0 = np.asarray(inputs["W_h0"], f32); b_h0 = np.asarray(inputs["b_h0"], f32)
    W_c0 = np.asarray(inputs["W_c0"], f32); b_c0 = np.asarray(inputs["b_c0"], f32)

    # gate column order per core: (i, f, o, g) chunks of 128
    def gate_cols(k):
        return np.concatenate([
            np.arange(k * 128, (k + 1) * 128),
            np.arange(1024 + k * 128, 1024 + (k + 1) * 128),
            np.arange(3072 + k * 128, 3072 + (k + 1) * 128),
            np.arange(2048 + k * 128, 2048 + (k + 1) * 128),
        ])

    q = np.arange(128)
    n_half = [(8 * h + q // 16) * 16 + q % 16 for h in range(2)]   # einsum row perm
    p_att = np.arange(128)
    nf_p, bl_p = p_att // 8, p_att % 8                   # attention partition fold

    emb = emb_table[captions[:, :TS]]                    # [64,31,512]
    embT = emb.transpose(2, 1, 0).reshape(EMB, TS * B)   # [512,(t,b)] t-major
    embT_aug = np.zeros((5 * 128, 2048), f32)
    embT_aug[:EMB, :TS * B] = embT
    embT_aug[EMB, :TS * B] = 1.0
    bias_g = b_ih + b_hh

    # fT_aug col c = nr*128 + nf*8 + bl, value F[bl, nr*16+nf, :]
    cc = np.arange(2048)
    nr_c, nf_c, bl_c = cc // 128, (cc % 128) // 8, cc % 8
    n_c = nr_c * 16 + nf_c
    vc_ = n_c < NF

    wua = np.zeros((17 * 128, ATTN), f32)
    wua[:ENC_H] = W_u
    wua[ENC_H] = b_u + b_w

    mean_blk = np.zeros((128, 16, 8), f32)
    for kt in range(16):
        bl, h = kt // 2, kt % 2
        nn = n_half[h]
        mean_blk[nn < NF, kt, bl] = 1.0 / NF
    ones_blk = np.zeros((128, 8), f32)
    ones_blk[p_att, bl_p] = 1.0
    onesb = np.zeros((128, B), f32)
    onesb[0, :] = 1.0
    # mask over (p=(nf,bl), nr): valid iff nr*16+nf < 196
    mask = ((np.arange(NR)[None, :] * 16 + nf_p[:, None]) < NF).astype(f32)
    ident = np.eye(128, dtype=f32)

    in_maps = []
    for k in range(NC):
        fb = features[k * BL:(k + 1) * BL]               # [8,196,2048]
        fe = np.zeros((128, 16, ENC_H), f32)
        for kt in range(16):
            bl, h = kt // 2, kt % 2
            nn = n_half[h]
            msk = nn < NF
            fe[msk, kt, :] = fb[bl, nn[msk], :]
        fta = np.zeros((17 * 128, 2048), f32)
        fta[:ENC_H, vc_] = fb[bl_c[vc_], n_c[vc_], :].T
        fta[ENC_H, :] = 1.0
        cols = gate_cols(k)
        wtop = np.zeros((5 * 128, CS), f32)
        wtop[:EMB] = W_ih[:EMB, cols]
        wtop[EMB] = bias_g[cols]
        wh0a = np.zeros((17 * 128, HS), f32)
        wh0a[:ENC_H] = W_h0[:, k * HS:(k + 1) * HS]
        wh0a[ENC_H] = b_h0[k * HS:(k + 1) * HS]
        wc0a = np.zeros((17 * 128, HS), f32)
        wc0a[:ENC_H] = W_c0[:, k * HS:(k + 1) * HS]
        wc0a[ENC_H] = b_c0[k * HS:(k + 1) * HS]
        bsel_np = np.zeros((128, 128), f32)
        bsel_np[k * BL + bl_p, p_att] = 1.0

        in_maps.append({
            "f_ein": fe.reshape(128, 16 * ENC_H).astype(bf16),
            "fT_aug": fta.astype(bf16),
            "W_u_aug": wua.astype(bf16),
            "embT_aug": embT_aug.astype(bf16),
            "W_top_aug": wtop.astype(bf16),
            "W_mid": W_ih[EMB:, cols].astype(bf16),
            "W_hh": W_hh[:, cols].astype(bf16),
            "W_w": W_w.astype(bf16),
            "W_a_rep": np.tile(W_a[:, 0][None, :], (128, 1)).astype(bf16),
            "W_h0_aug": wh0a.astype(bf16),
            "W_c0_aug": wc0a.astype(bf16),
            "w_fc": W_fc[:, k * VS:(k + 1) * VS].astype(bf16),
            "mean_blk": mean_blk.reshape(128, 128).astype(bf16),
            "bsel": bsel_np.astype(bf16),
            "ones_blk": ones_blk.astype(bf16),
            "onesb": onesb.astype(bf16),
            "mask": mask.astype(bf16),
            "zer": np.zeros((128, 512), bf16),
            "ident": ident.astype(bf16),
        })
    return in_maps


def assemble(results, inputs):
    b_fc = np.asarray(inputs["b_fc"], np.float32)
    preds = np.empty((B, TS, VOCAB), np.float32)
    attn = np.empty((B, TS, NF), np.float32)
    for k in range(NC):
        pp = results[k]["preds_part"]                    # [2048, 4000] rows (t,b)
        preds[:, :, k * VS:(k + 1) * VS] = (
            pp[:TS * B].reshape(TS, B, VS).transpose(1, 0, 2))
        scr = results[k]["attn_scr"]                     # [31,128,16] = [t,(nf,bl),nr]
        rec = results[k]["rec_scr"]                      # [31,8,1]
        a = scr.reshape(TS, 16, 8, NR)                   # [t, nf, bl, nr]
        a = a.transpose(2, 0, 3, 1).reshape(8, TS, 256)[:, :, :NF]  # n = nr*16+nf
        a = a * rec[:, :, 0].T[:, :, None]               # [bl, t, 1] normalize
        attn[k * BL:(k + 1) * BL] = a
    if np.any(b_fc != 0):
        preds += b_fc[None, None, :]
    return preds, attn


def kernel(**inputs):
    in_maps = host_prep(inputs)
    nc = build()
    res = run_bass_kernel_spmd(nc, in_maps, list(range(NC)))
    return assemble(res.results, inputs)


if __name__ == "__main__":
    build()
    print("build ok")


# revision 9
# speedup vs baseline: 1.0074x; 1.0074x over previous
"""Trainium2 Bass kernel for nn_DecoderRNN (attention LSTM decoder).

Sharding (8 cores):
  - attention (tanh/softmax DVE/ACT work + context einsum): batch-sharded (8/core)
  - LSTM gates matmul: gate-column-sharded (512 cols/core, 128 per gate);
    h/c column-sharded 128/core
  - fc_out: vocab-sharded (4000 cols/core)
Per step two AllGather exchanges: context [64,2048] and hT [1024,64].
Softmax normalization is deferred: the context einsum uses unnormalized
exp(scores); the 1/sum scale is applied at the context psum evacuation and
attention weights are normalized on the host.

Host does: slicing/padding/transposes/casts of inputs, embedding gather,
final output assembly. All FLOPs run on device.
"""
import sys
sys.path.insert(0, "/opt/trn_rl_repo")
from contextlib import ExitStack
import numpy as np
import ml_dtypes

bf16 = ml_dtypes.bfloat16

import concourse.bass as bass
import concourse.bacc as bacc
import concourse.mybir as mybir
from concourse import tile
from concourse.bass_utils import run_bass_kernel_spmd

F32 = mybir.dt.float32
BF16 = mybir.dt.bfloat16
ADD = mybir.AluOpType.add
MUL = mybir.AluOpType.mult
TANH = mybir.ActivationFunctionType.Tanh
EXP = mybir.ActivationFunctionType.Exp

NC = 8                     # cores
B, NF, T = 64, 196, 32     # batch, n_feat, caption len
TS = T - 1                 # 31 decode steps
EMB, VOCAB, ATTN, ENC_H, DEC_H = 512, 32000, 512, 2048, 1024
BL = B // NC               # 8 local batches
CS = 4 * DEC_H // NC       # 512 gate cols per core (128 per gate)
HS = DEC_H // NC           # 128 h cols per core
VS = VOCAB // NC           # 4000 vocab cols per core
NR, NFo = 16, 16           # n = nr*16 + nf (pad 196 -> 256); partition p = nf*8+bl
AFREE = NR * ATTN          # 8192 attention free size

_nc_cache = None


def build():
    global _nc_cache
    if _nc_cache is not None:
        return _nc_cache
    nc = bacc.Bacc("TRN2", target_bir_lowering=False, debug=False, num_devices=NC)
    tc = tile.TileContext(nc)

    def inp(name, shape, dtype=BF16):
        return nc.declare_dram_parameter(name, shape, dtype, isOutput=False)

    f_ein = inp("f_ein", [128, 16 * ENC_H])          # einsum rhs tiles
    fT_aug = inp("fT_aug", [17 * 128, 2048])         # F^T (perm cols) + bias block
    W_u_aug = inp("W_u_aug", [17 * 128, ATTN])
    embT_aug = inp("embT_aug", [5 * 128, 2048])      # emb^T ((t,b) cols) + bias block
    W_top_aug = inp("W_top_aug", [5 * 128, CS])
    W_mid = inp("W_mid", [ENC_H, CS])                # W_ih[512:, cols]
    W_hh = inp("W_hh", [DEC_H, CS])
    W_w = inp("W_w", [DEC_H, ATTN])
    W_a_rep = inp("W_a_rep", [128, ATTN])
    W_h0_aug = inp("W_h0_aug", [17 * 128, HS])
    W_c0_aug = inp("W_c0_aug", [17 * 128, HS])
    w_fc = inp("w_fc", [DEC_H, VS])
    mean_blk = inp("mean_blk", [128, 128])           # 1/NF blockdiag for feat-mean
    bsel = inp("bsel", [128, 128])                   # local-batch selector (pad rows 0)
    ones_blk = inp("ones_blk", [128, 8])             # group-sum lhsT
    onesb = inp("onesb", [128, B])                   # row0=1 block (bias K-tile lhsT)
    mask = inp("mask", [128, NR])                    # 1.0 on valid n slots
    zer = inp("zer", [128, 512])                     # zeros
    ident = inp("ident", [128, 128])

    preds_part = nc.declare_dram_parameter("preds_part", [2048, VS], F32, isOutput=True)
    attn_scr = nc.declare_dram_parameter("attn_scr", [TS, 128, NR], F32, isOutput=True)
    rec_scr = nc.declare_dram_parameter("rec_scr", [TS, BL, 1], F32, isOutput=True)

    gr = [list(range(NC))]

    with ExitStack() as ctx:
        ctx.enter_context(tc)
        # ---------- pools: P lives for phase0+phase1 then closes before fc ----
        D = ctx.enter_context(tc.tile_pool(name="drams", bufs=1, space="DRAM"))
        p1ctx = ExitStack()
        P = p1ctx.enter_context(tc.tile_pool(name="persist", bufs=1))
        fein_sb = P.tile([128, 16 * ENC_H], BF16)
        Uf = P.tile([128, AFREE], BF16)
        g0 = P.tile([128, 16 * CS], BF16)            # gates0, rows (t,b) 2048pad
        Wmid_sb = P.tile([128, 16 * CS], BF16)
        Whh_sb = P.tile([128, 8 * CS], BF16)
        Ww_sb = P.tile([128, 8 * ATTN], BF16)
        Wa_sb = P.tile([128, ATTN], BF16)
        bsel_sb = P.tile([128, 128], BF16)
        ob_sb = P.tile([128, 8], BF16)
        onesb_sb = P.tile([128, B], BF16)
        mask_sb = P.tile([128, NR], BF16)
        id_sb = P.tile([128, 128], BF16)
        Ablk = P.tile([128, 16 * 8], BF16)           # einsum lhsT blockdiag
        hT_full = P.tile([128, 8 * B], BF16)         # [128,(8kt,64b)]
        pre_all = P.tile([128, ATTN], BF16)          # rows 0:64 written per step
        c_cs = P.tile([B, HS], F32)

        ctx_shard = D.tile([BL, ENC_H], BF16, tag="ctx_shard")
        ctx_full = D.tile([B, ENC_H], BF16, tag="ctx_full")
        hT_shard = D.tile([HS, B], BF16, tag="hT_shard")
        hT_dram = D.tile([(TS + 2) * DEC_H, B], BF16, tag="hT_dram")
        fm_shard = D.tile([BL, ENC_H], BF16, tag="fm_shard")
        fm_full = D.tile([B, ENC_H], BF16, tag="fm_full")

        nc.sync.dma_start(fein_sb[:], f_ein[:])
        nc.sync.dma_start(Wmid_sb[:].rearrange("p (k c) -> p k c", k=16),
                          W_mid.rearrange("(k p) c -> p k c", p=128))
        nc.sync.dma_start(Whh_sb[:].rearrange("p (k c) -> p k c", k=8),
                          W_hh.rearrange("(k p) c -> p k c", p=128))
        nc.sync.dma_start(Ww_sb[:].rearrange("p (k c) -> p k c", k=8),
                          W_w.rearrange("(k p) c -> p k c", p=128))
        nc.sync.dma_start(Wa_sb[:], W_a_rep[:])
        nc.sync.dma_start(bsel_sb[:], bsel[:])
        nc.sync.dma_start(ob_sb[:], ones_blk[:])
        nc.sync.dma_start(onesb_sb[:], onesb[:])
        nc.sync.dma_start(mask_sb[:], mask[:])
        nc.sync.dma_start(id_sb[:], ident[:])
        nc.sync.dma_start(Ablk[:], zer[:, 0:128])
        nc.sync.dma_start(pre_all[64:128, :], zer[0:64, :])

        # ---------- phase 0 ----------
        with tc.tile_pool(name="ph0", bufs=2) as p0, \
             tc.tile_pool(name="ph0w", bufs=1) as p0w, \
             tc.tile_pool(name="ph0ps", bufs=1, space="PSUM") as pp0:
            def acc(i, shape=(128, 512), dt=F32):
                return pp0.tile(list(shape), dt, tag=f"acc{i}", name=f"acc{i}")

            # gates0 = embT_aug^T @ W_top_aug  (rows (t,b), cols CS)
            wtop = p0w.tile([128, 5 * CS], BF16, tag="wtop")
            nc.sync.dma_start(wtop[:].rearrange("p (k c) -> p k c", k=5),
                              W_top_aug.rearrange("(k p) c -> p k c", p=128))
            for half in range(2):
                psg = [acc(m) for m in range(8)]
                for k in range(5):
                    et = p0.tile([128, 1024], BF16, tag="ldA", name="ldA")
                    nc.sync.dma_start(et[:], embT_aug[k * 128:(k + 1) * 128,
                                                      half * 1024:(half + 1) * 1024])
                    for m in range(8):
                        nc.tensor.matmul(psg[m][:], et[:, m * 128:(m + 1) * 128],
                                         wtop[:, k * CS:(k + 1) * CS],
                                         start=(k == 0), stop=(k == 4))
                for m in range(8):
                    mt = half * 8 + m
                    nc.vector.tensor_copy(g0[:, mt * CS:(mt + 1) * CS], psg[m][:])

            # Uf = fT_aug^T @ W_u_aug  (m-tile == nr chunk)
            for half in range(2):
                psu = [acc(m) for m in range(8)]
                for k in range(17):
                    ft = p0.tile([128, 1024], BF16, tag="ldA", name="ldA")
                    nc.sync.dma_start(ft[:], fT_aug[k * 128:(k + 1) * 128,
                                                    half * 1024:(half + 1) * 1024])
                    wu = p0.tile([128, ATTN], BF16, tag="ldB", name="ldB")
                    nc.sync.dma_start(wu[:], W_u_aug[k * 128:(k + 1) * 128, :])
                    for m in range(8):
                        nc.tensor.matmul(psu[m][:], ft[:, m * 128:(m + 1) * 128],
                                         wu[:], start=(k == 0), stop=(k == 16))
                for m in range(8):
                    nr = half * 8 + m
                    nc.vector.tensor_copy(Uf[:, nr * ATTN:(nr + 1) * ATTN], psu[m][:])

            # feat mean via einsum with mean_blk
            mb = p0.tile([128, 128], BF16, tag="ldB", name="ldB")
            nc.sync.dma_start(mb[:], mean_blk[:])
            psf = [acc(c, (BL, 512)) for c in range(4)]
            for kt in range(16):
                for c in range(4):
                    nc.tensor.matmul(psf[c][:], mb[:, kt * 8:(kt + 1) * 8],
                                     fein_sb[:, kt * ENC_H + c * 512:kt * ENC_H + (c + 1) * 512],
                                     start=(kt == 0), stop=(kt == 15))
            fmb = p0.tile([BL, ENC_H], BF16, tag="fmb")
            for c in range(4):
                nc.vector.tensor_copy(fmb[:, c * 512:(c + 1) * 512], psf[c][:])
            nc.sync.dma_start(fm_shard[:], fmb[:])
            nc.gpsimd.collective_compute("AllGather", mybir.AluOpType.bypass,
                                         replica_groups=gr, ins=[fm_shard[:].opt()],
                                         outs=[fm_full[:].opt()])
            fmf = p0.tile([B, ENC_H], BF16, tag="fmf")
            nc.sync.dma_start(fmf[:], fm_full[:])
            fmT = p0.tile([128, 16 * B], BF16, tag="fmT")
            for k in range(16):
                pst = acc(4, (128, B), BF16)
                nc.tensor.transpose(pst[:], fmf[:, k * 128:(k + 1) * 128], id_sb[:B, :B])
                nc.vector.tensor_copy(fmT[:, k * B:(k + 1) * B], pst[:])
            # h0/c0 col slices
            wh0 = p0w.tile([128, 17 * HS], BF16, tag="wh0")
            wc0 = p0w.tile([128, 17 * HS], BF16, tag="wc0")
            nc.sync.dma_start(wh0[:].rearrange("p (k c) -> p k c", k=17),
                              W_h0_aug.rearrange("(k p) c -> p k c", p=128))
            nc.sync.dma_start(wc0[:].rearrange("p (k c) -> p k c", k=17),
                              W_c0_aug.rearrange("(k p) c -> p k c", p=128))
            ph0 = acc(0, (B, HS))
            pc0 = acc(1, (B, HS))
            for k in range(17):
                lh = fmT[:, k * B:(k + 1) * B] if k < 16 else onesb_sb[:]
                nc.tensor.matmul(ph0[:], lh, wh0[:, k * HS:(k + 1) * HS],
                                 start=(k == 0), stop=(k == 16))
                nc.tensor.matmul(pc0[:], lh, wc0[:, k * HS:(k + 1) * HS],
                                 start=(k == 0), stop=(k == 16))
            nc.vector.tensor_copy(c_cs[:], pc0[:])
            h_bf0 = p0.tile([B, HS], BF16, tag="hbf0")
            nc.vector.tensor_copy(h_bf0[:], ph0[:])
            psT0 = acc(2, (HS, B), BF16)
            nc.tensor.transpose(psT0[:], h_bf0[:], id_sb[:B, :B])
            hsh0 = p0.tile([HS, B], BF16, tag="hsh0")
            nc.vector.tensor_copy(hsh0[:], psT0[:])
            nc.sync.dma_start(hT_shard[:], hsh0[:])
            nc.gpsimd.collective_compute("AllGather", mybir.AluOpType.bypass,
                                         replica_groups=gr, ins=[hT_shard[:].opt()],
                                         outs=[hT_dram[0:DEC_H, :].opt()])
            nc.sync.dma_start(
                hT_full[:].rearrange("p (k b) -> p k b", k=8),
                hT_dram[0:DEC_H, :].rearrange("(k p) b -> p k b", p=128))

        # ---------- phase 1: 31 steps ----------
        with tc.tile_pool(name="step", bufs=2) as sp, \
             tc.tile_pool(name="stepps", bufs=3, space="PSUM") as ps, \
             tc.tile_pool(name="ctxps", bufs=1, space="PSUM") as psc:
            for t in range(TS):
                # --- pre = h @ W_w for ALL b, then select+replicate local 8 ---
                pre_ps = ps.tile([B, ATTN], F32, tag="mm", name="pre_ps")
                for k in range(8):
                    nc.tensor.matmul(pre_ps[:], hT_full[:, k * B:(k + 1) * B],
                                     Ww_sb[:, k * ATTN:(k + 1) * ATTN],
                                     start=(k == 0), stop=(k == 7))
                nc.vector.tensor_copy(pre_all[0:B, :], pre_ps[:])
                prr_ps = ps.tile([128, ATTN], F32, tag="mm", name="prr_ps")
                nc.tensor.matmul(prr_ps[:], bsel_sb[:], pre_all[:], start=True, stop=True)
                pre_rep = sp.tile([128, ATTN], BF16, tag="pre_rep")
                nc.vector.tensor_copy(pre_rep[:], prr_ps[:])
                # --- att = tanh(Uf + pre) ; scores = sum_a att*W_a ---
                att = sp.tile([128, AFREE], BF16, tag="att")
                nc.vector.tensor_tensor(
                    att[:].rearrange("p (r a) -> p r a", r=NR),
                    Uf[:].rearrange("p (r a) -> p r a", r=NR),
                    bass.AP(pre_rep[:].tensor, 0, [[ATTN, 128], [0, NR], [1, ATTN]]),
                    ADD)
                nc.scalar.activation(att[:], att[:], TANH)
                nc.vector.tensor_tensor(
                    att[:].rearrange("p (r a) -> p r a", r=NR),
                    att[:].rearrange("p (r a) -> p r a", r=NR),
                    bass.AP(Wa_sb[:].tensor, 0, [[ATTN, 128], [0, NR], [1, ATTN]]),
                    MUL)
                scores = sp.tile([128, NR], F32, tag="scores")
                nc.vector.tensor_reduce(scores[:],
                                        att[:].rearrange("p (r a) -> p r a", r=NR),
                                        mybir.AxisListType.X, ADD)
                # --- masked exp; deferred softmax normalization ---
                expv = sp.tile([128, NR], F32, tag="expv")
                nc.scalar.activation(expv[:], scores[:], EXP)
                exb = sp.tile([128, NR], BF16, tag="exb")
                nc.vector.tensor_tensor(exb[:], expv[:], mask_sb[:], MUL)
                ssum = ps.tile([BL, NR], F32, tag="mm", name="ssum")
                nc.tensor.matmul(ssum[:], ob_sb[:], exb[:], start=True, stop=True)
                ssum_sb = sp.tile([BL, NR], F32, tag="ssum_sb")
                nc.vector.tensor_copy(ssum_sb[:], ssum[:])
                stot = sp.tile([BL, 1], F32, tag="stot")
                nc.vector.tensor_reduce(stot[:], ssum_sb[:], mybir.AxisListType.X, ADD)
                rec = sp.tile([BL, 1], F32, tag="rec")
                nc.vector.reciprocal(rec[:], stot[:])
                nc.scalar.dma_start(attn_scr[t], expv[:])
                nc.scalar.dma_start(rec_scr[t], rec[:])
                # --- exp -> blockdiag lhsT (transpose + 2 scatter DMAs) ---
                aT_ps = ps.tile([NR, 128], BF16, tag="mm", name="aT_ps")
                nc.tensor.transpose(aT_ps[:], exb[:], id_sb[:])
                aT_sb = sp.tile([NR, 128], BF16, tag="aT_sb")
                nc.vector.tensor_copy(aT_sb[:], aT_ps[:])
                for h in range(2):
                    asrc = bass.AP(aT_sb[:].tensor, 8 * h * 128,
                                   [[128, 8], [8, NFo], [1, 8]])
                    adst = bass.AP(Ablk[:].tensor, 8 * h, [[128, 128], [17, 8]])
                    nc.sync.dma_start(adst, asrc)
                # --- context einsum (unnormalized), scaled on evacuation ---
                ctx_ps = psc.tile([BL, ENC_H], F32, tag="ctx_ps", name="ctx_ps")
                for kt in range(16):
                    for c in range(4):
                        nc.tensor.matmul(
                            ctx_ps[:, c * 512:(c + 1) * 512],
                            Ablk[:, kt * 8:(kt + 1) * 8],
                            fein_sb[:, kt * ENC_H + c * 512:kt * ENC_H + (c + 1) * 512],
                            start=(kt == 0), stop=(kt == 15))
                ctx_bf = sp.tile([BL, ENC_H], BF16, tag="ctx_bf")
                for c in range(2):
                    nc.vector.tensor_scalar(ctx_bf[:, c * 1024:(c + 1) * 1024],
                                            ctx_ps[:, c * 1024:(c + 1) * 1024],
                                            rec[:], None, MUL)
                nc.sync.dma_start(ctx_shard[:], ctx_bf[:])
                nc.gpsimd.collective_compute("AllGather", mybir.AluOpType.bypass,
                                             replica_groups=gr, ins=[ctx_shard[:].opt()],
                                             outs=[ctx_full[:].opt()])
                ctxf = sp.tile([B, ENC_H], BF16, tag="ctxf")
                nc.scalar.dma_start(ctxf[:], ctx_full[:])
                ctxT = sp.tile([128, 16 * B], BF16, tag="ctxT")
                for k4 in range(4):
                    cT_ps = ps.tile([128, 4 * B], BF16, tag="mm", name="cT_ps")
                    for j in range(4):
                        kk = k4 * 4 + j
                        nc.tensor.transpose(cT_ps[:, j * B:(j + 1) * B],
                                            ctxf[:, kk * 128:(kk + 1) * 128],
                                            id_sb[:B, :B])
                    nc.vector.tensor_copy(ctxT[:, k4 * 4 * B:(k4 + 1) * 4 * B], cT_ps[:])
                # --- gates ---
                g_ps = ps.tile([B, CS], F32, tag="mm", name="g_ps")
                for k in range(16):
                    nc.tensor.matmul(g_ps[:], ctxT[:, k * B:(k + 1) * B],
                                     Wmid_sb[:, k * CS:(k + 1) * CS],
                                     start=(k == 0), stop=False)
                for k in range(8):
                    nc.tensor.matmul(g_ps[:], hT_full[:, k * B:(k + 1) * B],
                                     Whh_sb[:, k * CS:(k + 1) * CS],
                                     start=False, stop=(k == 7))
                gat = sp.tile([B, CS], F32, tag="gat")
                nc.vector.tensor_tensor(gat[:], g_ps[:],
                                        g0[(t % 2) * 64:(t % 2) * 64 + 64,
                                           (t // 2) * CS:(t // 2 + 1) * CS], ADD)
                # --- elementwise: col chunks (i, f, o, g) of 128 ---
                sig = sp.tile([B, 384], F32, tag="sig")
                nc.scalar.activation(sig[:], gat[:, 0:384], TANH, scale=0.5)
                nc.vector.tensor_scalar(sig[:], sig[:], 0.5, 0.5, MUL, ADD)
                tg = sp.tile([B, HS], F32, tag="tg")
                nc.scalar.activation(tg[:], gat[:, 384:512], TANH)
                m1 = sp.tile([B, HS], F32, tag="m1")
                nc.vector.tensor_tensor(m1[:], sig[:, 128:256], c_cs[:], MUL)
                m2 = sp.tile([B, HS], F32, tag="m2")
                nc.vector.tensor_tensor(m2[:], sig[:, 0:128], tg[:], MUL)
                nc.vector.tensor_tensor(c_cs[:], m1[:], m2[:], ADD)
                tcn = sp.tile([B, HS], F32, tag="tcn")
                nc.scalar.activation(tcn[:], c_cs[:], TANH)
                h_bf = sp.tile([B, HS], BF16, tag="h_bf")
                nc.vector.tensor_tensor(h_bf[:], sig[:, 256:384], tcn[:], MUL)
                # --- h exchange ---
                hT_ps = ps.tile([HS, B], BF16, tag="mm", name="hT_ps")
                nc.tensor.transpose(hT_ps[:], h_bf[:], id_sb[:B, :B])
                hsh = sp.tile([HS, B], BF16, tag="hsh")
                nc.vector.tensor_copy(hsh[:], hT_ps[:])
                nc.sync.dma_start(hT_shard[:], hsh[:])
                nc.gpsimd.collective_compute(
                    "AllGather", mybir.AluOpType.bypass, replica_groups=gr,
                    ins=[hT_shard[:].opt()],
                    outs=[hT_dram[(t + 1) * DEC_H:(t + 2) * DEC_H, :].opt()])
                nc.scalar.dma_start(
                    hT_full[:].rearrange("p (k b) -> p k b", k=8),
                    hT_dram[(t + 1) * DEC_H:(t + 2) * DEC_H, :]
                    .rearrange("(k p) b -> p k b", p=128))

        p1ctx.close()
        # ---------- fc phase: everything SBUF-resident ----------
        with tc.tile_pool(name="fc", bufs=1) as fp, \
             tc.tile_pool(name="fcev", bufs=2) as fe, \
             tc.tile_pool(name="fcps", bufs=1, space="PSUM") as fps:
            NVC = 8
            VC = VS // NVC  # 500
            hT_view = hT_dram[:].rearrange("(t p) b -> t p b", p=DEC_H)
            HT_sb = fp.tile([128, 8 * 2048], BF16, name="HT_sb")
            for k in range(8):
                eng = nc.sync if k % 2 == 0 else nc.scalar
                eng.dma_start(
                    HT_sb[:, k * 2048:(k + 1) * 2048].rearrange("p (t b) -> p t b", t=32),
                    hT_view[1:33, k * 128:(k + 1) * 128, :].rearrange("t p b -> p t b"))
            wfc_sb = fp.tile([128, 8 * VS], BF16, name="wfc_sb")
            nc.sync.dma_start(wfc_sb[:].rearrange("p (k v) -> p k v", k=8),
                              w_fc.rearrange("(k p) v -> p k v", p=128))
            for half in range(2):
                for vc in range(NVC):
                    psv = [fps.tile([128, VC], F32, tag=f"psv{m}", name=f"psv{m}")
                           for m in range(8)]
                    for k in range(8):
                        for m in range(8):
                            mt = half * 8 + m
                            nc.tensor.matmul(
                                psv[m][:],
                                HT_sb[:, k * 2048 + mt * 128:k * 2048 + (mt + 1) * 128],
                                wfc_sb[:, k * VS + vc * VC:k * VS + (vc + 1) * VC],
                                start=(k == 0), stop=(k == 7))
                    ev = fe.tile([128, 8 * VC], F32, tag="ev", name="ev")
                    for m in range(8):
                        if m % 2 == 0:
                            nc.vector.tensor_copy(ev[:, m * VC:(m + 1) * VC], psv[m][:])
                        else:
                            nc.scalar.copy(ev[:, m * VC:(m + 1) * VC], psv[m][:])
                    pdst = bass.AP(preds_part, half * 8 * 128 * VS + vc * VC,
                                   [[VS, 128], [128 * VS, 8], [1, VC]])
                    esrc = bass.AP(ev[:].tensor, 0, [[8 * VC, 128], [VC, 8], [1, VC]])
                    eng = nc.sync if vc % 2 == 0 else nc.scalar
                    eng.dma_start(pdst, esrc)
    nc.finalize()
    _nc_cache = nc
    return nc


def host_prep(inputs):
    f32 = np.float32
    features = np.asarray(inputs["features"], f32)       # [64,196,2048]
    captions = np.asarray(inputs["captions"])            # [64,32] int
    emb_table = np.asarray(inputs["emb_table"], f32)
    W_u = np.asarray(inputs["W_u"], f32); b_u = np.asarray(inputs["b_u"], f32)
    W_w = np.asarray(inputs["W_w"], f32); b_w = np.asarray(inputs["b_w"], f32)
    W_a = np.asarray(inputs["W_a"], f32)
    W_ih = np.asarray(inputs["W_ih"], f32); b_ih = np.asarray(inputs["b_ih"], f32)
    W_hh = np.asarray(inputs["W_hh"], f32); b_hh = np.asarray(inputs["b_hh"], f32)
    W_fc = np.asarray(inputs["W_fc"], f32)
    W_h0 = np.asarray(inputs["W_h0"], f32); b_h0 = np.asarray(inputs["b_h0"], f32)
    W_c0 = np.asarray(inputs["W_c0"], f32); b_c0 = np.asarray(inputs["b_c0"], f32)

    # gate column order per core: (i, f, o, g) chunks of 128
    def gate_cols(k):
        return np.concatenate([
            np.arange(k * 128, (k + 1) * 128),
            np.arange(1024 + k * 128, 1024 + (k + 1) * 128),
            np.arange(3072 + k * 128, 3072 + (k + 1) * 128),
            np.arange(2048 + k * 128, 2048 + (k + 1) * 128),
        ])

    q = np.arange(128)
    n_half = [(8 * h + q // 16) * 16 + q % 16 for h in range(2)]   # einsum row perm
    p_att = np.arange(128)
    nf_p, bl_p = p_att // 8, p_att % 8                   # attention partition fold

    emb = emb_table[captions[:, :TS]]                    # [64,31,512]
    embT = emb.transpose(2, 1, 0).reshape(EMB, TS * B)   # [512,(t,b)] t-major
    embT_aug = np.zeros((5 * 128, 2048), f32)
    embT_aug[:EMB, :TS * B] = embT
    embT_aug[EMB, :TS * B] = 1.0
    bias_g = b_ih + b_hh

    # fT_aug col c = nr*128 + nf*8 + bl, value F[bl, nr*16+nf, :]
    cc = np.arange(2048)
    nr_c, nf_c, bl_c = cc // 128, (cc % 128) // 8, cc % 8
    n_c = nr_c * 16 + nf_c
    vc_ = n_c < NF

    wua = np.zeros((17 * 128, ATTN), f32)
    wua[:ENC_H] = W_u
    wua[ENC_H] = b_u + b_w

    mean_blk = np.zeros((128, 16, 8), f32)
    for kt in range(16):
        bl, h = kt // 2, kt % 2
        nn = n_half[h]
        mean_blk[nn < NF, kt, bl] = 1.0 / NF
    ones_blk = np.zeros((128, 8), f32)
    ones_blk[p_att, bl_p] = 1.0
    onesb = np.zeros((128, B), f32)
    onesb[0, :] = 1.0
    # mask over (p=(nf,bl), nr): valid iff nr*16+nf < 196
    mask = ((np.arange(NR)[None, :] * 16 + nf_p[:, None]) < NF).astype(f32)
    ident = np.eye(128, dtype=f32)

    in_maps = []
    for k in range(NC):
        fb = features[k * BL:(k + 1) * BL]               # [8,196,2048]
        fe = np.zeros((128, 16, ENC_H), f32)
        for kt in range(16):
            bl, h = kt // 2, kt % 2
            nn = n_half[h]
            msk = nn < NF
            fe[msk, kt, :] = fb[bl, nn[msk], :]
        fta = np.zeros((17 * 128, 2048), f32)
        fta[:ENC_H, vc_] = fb[bl_c[vc_], n_c[vc_], :].T
        fta[ENC_H, :] = 1.0
        cols = gate_cols(k)
        wtop = np.zeros((5 * 128, CS), f32)
        wtop[:EMB] = W_ih[:EMB, cols]
        wtop[EMB] = bias_g[cols]
        wh0a = np.zeros((17 * 128, HS), f32)
        wh0a[:ENC_H] = W_h0[:, k * HS:(k + 1) * HS]
        wh0a[ENC_H] = b_h0[k * HS:(k + 1) * HS]
        wc0a = np.zeros((17 * 128, HS), f32)
        wc0a[:ENC_H] = W_c0[:, k * HS:(k + 1) * HS]
        wc0a[ENC_H] = b_c0[k * HS:(k + 1) * HS]
        bsel_np = np.zeros((128, 128), f32)
        bsel_np[k * BL + bl_p, p_att] = 1.0

        in_maps.append({
            "f_ein": fe.reshape(128, 16 * ENC_H).astype(bf16),
            "fT_aug": fta.astype(bf16),
            "W_u_aug": wua.astype(bf16),
            "embT_aug": embT_aug.astype(bf16),
            "W_top_aug": wtop.astype(bf16),
            "W_mid": W_ih[EMB:, cols].astype(bf16),
            "W_hh": W_hh[:, cols].astype(bf16),
            "W_w": W_w.astype(bf16),
            "W_a_rep": np.tile(W_a[:, 0][None, :], (128, 1)).astype(bf16),
            "W_h0_aug": wh0a.astype(bf16),
            "W_c0_aug": wc0a.astype(bf16),
            "w_fc": W_fc[:, k * VS:(k + 1) * VS].astype(bf16),
            "mean_blk": mean_blk.reshape(128, 128).astype(bf16),
            "bsel": bsel_np.astype(bf16),
            "ones_blk": ones_blk.astype(bf16),
            "onesb": onesb.astype(bf16),
            "mask": mask.astype(bf16),
            "zer": np.zeros((128, 512), bf16),
            "ident": ident.astype(bf16),
        })
    return in_maps


def assemble(results, inputs):
    b_fc = np.asarray(inputs["b_fc"], np.float32)
    preds = np.empty((B, TS, VOCAB), np.float32)
    attn = np.empty((B, TS, NF), np.float32)
    for k in range(NC):
        pp = results[k]["preds_part"]                    # [2048, 4000] rows (t,b)
        preds[:, :, k * VS:(k + 1) * VS] = (
            pp[:TS * B].reshape(TS, B, VS).transpose(1, 0, 2))
        scr = results[k]["attn_scr"]                     # [31,128,16] = [t,(nf,bl),nr]
        rec = results[k]["rec_scr"]                      # [31,8,1]
        a = scr.reshape(TS, 16, 8, NR)                   # [t, nf, bl, nr]
        a = a.transpose(2, 0, 3, 1).reshape(8, TS, 256)[:, :, :NF]  # n = nr*16+nf
        a = a * rec[:, :, 0].T[:, :, None]               # [bl, t, 1] normalize
        attn[k * BL:(k + 1) * BL] = a
    if np.any(b_fc != 0):
        preds += b_fc[None, None, :]
    return preds, attn


def kernel(**inputs):
    in_maps = host_prep(inputs)
    nc = build()
    res = run_bass_kernel_spmd(nc, in_maps, list(range(NC)))
    return assemble(res.results, inputs)


if __name__ == "__main__":
    build()
    print("build ok")
